# revision 27
# baseline (speedup 1.0000x reference)
"""Trainium2 Bass kernel for nn_MultiHeadAttention_70497593196785.

Causal MHA with q=k=v=x.view(B,L,H,DH) (the projections are dead code in the
reference). Returns (out, attn) where out is (B, DH, L*H) — a pure reshape of
the (B, L, H, DH) attention output — and attn is the full (B, H, L, L) softmax
matrix.

Sharding: B*H = 64 (batch, head) pairs; core c owns batch c//2 and heads
[(c%2)*8, (c%2)*8+8). Attention is independent per (b, h): no collectives.

Per-core algorithm, per head:
  - xT (64, L) via PE transposes of x blocks; xv = [x | ones] (128-row blocks).
  - For each k-block kb: scoresT[kb] = (xT[:,kb].T @ xT)  (k on partitions,
    q free, lower-triangular region only), exp via ACT (scale=1/8) into an
    SBUF strip, causal masking of the diagonal block via a triu multiply.
  - PV: out.T chunks (65, 512) accumulate xv[kb].T @ strip[kb] over kb; the
    ones column yields the softmax denominator Z per q (flash-style late
    normalization).
  - Per q-block: transpose out.T chunk -> (128, 65), rz = 1/Z, write out rows;
    PE-transpose strip blocks back to (q, k) layout and scale by rz while
    copying PSUM->SBUF (split between DVE and ACT), DMA attn rows.
  - attn's upper triangle is never written: output buffers are zero-initialized
    by the runtime (donated zero buffers), matching exp(-1e9) == 0 exactly.

fp32r (tf32-class PE rounding, ~6e-5 rel err measured, full PE speed) is used
for both matmul stages by default; transposes are plain fp32 (exact).
"""

import math
import os
from functools import lru_cache

import numpy as np

import concourse.bacc as bacc
import concourse.mybir as mybir
import concourse.tile as tile
from concourse.bass_utils import run_bass_kernel_spmd
from concourse.masks import make_identity

f32 = mybir.dt.float32
f32r = mybir.dt.float32r
i32 = mybir.dt.int32
EXP = mybir.ActivationFunctionType.Exp

B, L, D = 4, 2048, 1024
H, DH = 16, 64
P = 128
NH = 8            # heads per core
KB = L // P       # 16 k-blocks
CH = 512          # PV chunk width
NCH = L // CH     # 4 chunks
SCALE = 1.0 / math.sqrt(DH)
N_CORES = 8

# dtype knobs: f32r = fast (tf32-class rounding in PE), f32 = exact, 4x slower
SC_DT = f32r      # QK^T scores matmul
PV_DT = f32r      # attn @ V matmul
# fraction (out of 10) of transposed-attn groups normalized on ScalarE
# instead of VectorE, to balance the two engines
ACT_NORM_TENTHS = 3
SPSUM_COLS = 1024  # exp window width (PSUM banks per S tile = COLS/512)
SPSUM_BUFS = 2
NPOP = 8           # deferred epilogue pieces popped per score slot
XT_NPOP = 0        # pops during the head-start xT phase (hurts: delays xT->scores)
PV_TRIM = True     # skip the zeroed left region of same-chunk PV streams
ROWP_BUFS = 6
ZEROS_ENGINE = "vector"   # or "gpsimd"
TRIU_ENGINE = "vector"    # or "gpsimd"


def _build(ablate=()):
    """ablate: subset of {"attn_dma", "norm", "attn_tr", "exp", "out_dma",
    "scores", "pv"} — timing experiments only (results become wrong)."""
    nc = bacc.Bacc(None, target_bir_lowering=False)
    x_sh = nc.dram_tensor("x_sh", [L, NH * DH], f32, kind="ExternalInput")
    mask_blk = nc.dram_tensor("mask_blk", [P, P], i32, kind="ExternalInput")
    attn_sh = nc.dram_tensor("attn_sh", [NH, L, L], f32, kind="ExternalOutput")
    out_sh = nc.dram_tensor("out_sh", [L, NH, DH], f32, kind="ExternalOutput")

    strip_dt = f32r if (PV_DT == f32r or SC_DT == f32r) else f32

    with tile.TileContext(nc) as tc:
        with (
            tc.tile_pool(name="const", bufs=1) as const,
            tc.tile_pool(name="xvp", bufs=2) as xvp,
            tc.tile_pool(name="xtp", bufs=2) as xtp,
            tc.tile_pool(name="strips", bufs=1) as strips_pool,
            tc.tile_pool(name="spsum", bufs=SPSUM_BUFS, space="PSUM") as spsum,
            tc.tile_pool(name="pvpsum", bufs=1, space="PSUM") as pvpsum,
            tc.tile_pool(name="tpsum", bufs=3, space="PSUM") as tpsum,
            tc.tile_pool(name="rowp", bufs=ROWP_BUFS) as rowp,
            tc.tile_pool(name="otp", bufs=2) as otp,
            tc.tile_pool(name="rzp", bufs=8) as rzp,
            tc.tile_pool(name="outsb", bufs=4) as outsb,
        ):
            ident = const.tile([P, P], f32)
            make_identity(nc, ident)

            # causal masks for the diagonal 128-blocks, from the input mask.
            # tril = mask[0:128, 0:128]; triu (its transpose) masks the
            # diagonal block in (k, q) layout.
            mi = const.tile([P, P], i32, tag="mi")
            nc.sync.dma_start(mi[:], mask_blk[:])
            maskf = const.tile([P, P], f32, tag="maskf")
            nc.vector.tensor_copy(maskf[:], mi[:])
            tmq = tpsum.tile([P, CH], f32, tag="t")
            nc.tensor.transpose(tmq[:, 0:P], maskf[:], ident[:])
            triu = const.tile([P, P], f32, tag="triu")
            nc.vector.tensor_copy(triu[:], tmq[:, 0:P])

            ones_kb = const.tile([P, KB], f32, tag="ones_kb")
            nc.vector.memset(ones_kb[:], 1.0)
            zeros_t = const.tile([P, CH], f32, tag="zeros_t")
            nc.vector.memset(zeros_t[:], 0.0)

            norm_ctr = [0]
            pend = []  # (head, strip_group, closure)
            xv_tiles = {}

            def load_xv(h):
                # Prefetched on the ScalarE HWDGE queue so it is not stuck
                # behind the 1MB attn-row writes on the SP queue.
                xv = xvp.tile([P, KB, DH + 1], PV_DT, tag="xv", name="xv")
                src = x_sh[:].rearrange("(ko p) d -> p ko d", p=P)[
                    :, :, h * DH : (h + 1) * DH
                ]
                if PV_DT != f32:
                    src = src.bitcast(PV_DT)
                nc.scalar.dma_start(xv[:, :, 0:DH], src)
                nc.vector.tensor_copy(xv[:, :, DH], ones_kb[:])
                xv_tiles[h] = xv

            load_xv(0)
            for h in range(NH):
                xv = xv_tiles.pop(h)

                # ---- xT (64, L) via PE transposes, 4 blocks per PSUM tile
                xt = xtp.tile([DH, L], SC_DT, tag="xt")
                for g in range(KB // 4):
                    for _ in range(min(len(pend), XT_NPOP)):
                        pend.pop(0)[2]()
                    tp = tpsum.tile([P, CH], f32, tag="t")
                    for j in range(4):
                        kb = 4 * g + j
                        nc.tensor.transpose(
                            tp[0:DH, j * P : (j + 1) * P],
                            xv[:, kb, 0:DH].bitcast(f32),
                            ident[:],
                        )
                    nc.vector.tensor_copy(
                        xt[:, g * CH : (g + 1) * CH], tp[0:DH, :]
                    )

                # ---- main loop over k-blocks
                strips = []  # (tile, base_col)
                ot = otp.tile([DH + 1, L], f32, tag="ot")
                for kb in range(KB):
                    # Interleave deferred attn-row work with the score
                    # matmuls so PE alternates between PSUM-gated transposes
                    # and deeply-pipelined matmuls. Items from the previous
                    # head MUST pop before the strip tag they read is
                    # rewritten (item g covers strips 4g..4g+3, rewritten by
                    # scores kb>=4g of the next head).
                    nmust = sum(
                        1 for it in pend if it[0] != h and it[1] * 4 <= kb
                    )
                    npop = max(nmust, min(len(pend), NPOP))
                    for _ in range(npop):
                        pend.pop(0)[2]()

                    if kb == 4 and h + 1 < NH:
                        load_xv(h + 1)

                    c0 = kb // 4
                    base = CH * c0
                    slen = L - base
                    st = strips_pool.tile([P, slen], strip_dt, tag=f"st{kb}")
                    lhs = xt[:, kb * P : (kb + 1) * P]
                    off = kb * P - base
                    # scores + exp over [kb*P, L) only (cols left of the
                    # diagonal block are masked anyway), in <=1024 col windows
                    j0 = off
                    while j0 < slen:
                        w = min(SPSUM_COLS, slen - j0)
                        S = spsum.tile([P, SPSUM_COLS], f32, tag="s")
                        for p0 in range(0, w, CH):
                            pw = min(CH, w - p0)
                            nc.tensor.matmul(
                                S[:, p0 : p0 + pw],
                                lhs,
                                xt[:, base + j0 + p0 : base + j0 + p0 + pw],
                                start=True,
                                stop=True,
                            )
                        if "exp" not in ablate:
                            nc.scalar.activation(
                                st[:, j0 : j0 + w], S[:, 0:w], EXP, scale=SCALE
                            )
                        j0 += w
                    # causal: zero columns left of the diagonal block, and
                    # apply the (transposed) triangular mask on the diagonal
                    if off:
                        getattr(nc, ZEROS_ENGINE).tensor_copy(
                            st[:, 0:off], zeros_t[:, 0:off]
                        )
                    getattr(nc, TRIU_ENGINE).tensor_tensor(
                        st[:, off : off + P],
                        st[:, off : off + P].bitcast(f32),
                        triu[:],
                        mybir.AluOpType.mult,
                    )
                    strips.append((st, base))

                    if kb % 4 != 3:
                        continue

                    # ---- PV for chunk c: out.T (65, CH) += xv[k2].T @ strip
                    c = kb // 4
                    pv = pvpsum.tile([P, CH], f32, tag="pv")
                    for k2 in [] if "pv" in ablate else range(kb + 1):
                        st2, b2 = strips[k2]
                        t0 = c * CH - b2
                        d0 = 0
                        if PV_TRIM and k2 // 4 == c and k2 % 4:
                            d0 = (k2 % 4) * P  # cols < k2*P are zero
                        rhs = st2[:, t0 + d0 : t0 + CH]
                        if PV_DT != strip_dt:
                            rhs = rhs.bitcast(PV_DT)
                        nc.tensor.matmul(
                            pv[0 : DH + 1, d0:CH],
                            xv[:, k2, :],
                            rhs,
                            start=(k2 == 0),
                            stop=(k2 == kb),
                        )
                    nc.vector.tensor_copy(
                        ot[:, c * CH : (c + 1) * CH], pv[0 : DH + 1, :]
                    )

                    # ---- per q-block epilogue for this chunk: the out rows
                    # are cheap and emitted inline; the attn-row transposes
                    # and normalizes are queued g-major (g = strip group) and
                    # interleaved into later score slots (see pop loop above).
                    rows = {}
                    for qi in range(4 * c, 4 * c + 4):
                        tq = tpsum.tile([P, CH], f32, tag="t")
                        nc.tensor.transpose(
                            tq[:, 0 : DH + 1],
                            ot[:, qi * P : (qi + 1) * P],
                            ident[0 : DH + 1, 0 : DH + 1],
                        )
                        rz = rzp.tile([P, 1], f32, tag="rz")
                        nc.vector.reciprocal(rz[:], tq[:, DH : DH + 1])
                        ob = outsb.tile([P, DH], f32, tag="ob")
                        nc.vector.tensor_scalar_mul(ob[:], tq[:, 0:DH], rz[:])
                        if "out_dma" not in ablate:
                            nc.sync.dma_start(
                                out_sh[qi * P : (qi + 1) * P, h, :], ob[:]
                            )
                        rows[qi] = (rowp.tile([P, L], f32, tag="row", name="row"), rz)

                    if "attn_tr" in ablate:
                        continue
                    for g in range(0, c + 1):
                        for qi in range(max(4 * c, 4 * g), 4 * c + 4):
                            def piece(g=g, qi=qi, h=h, strips=strips,
                                      row=rows[qi][0], rz=rows[qi][1]):
                                gw = min(4, qi + 1 - 4 * g)
                                tp = tpsum.tile([P, CH], f32, tag="t")
                                for j in range(gw):
                                    st2, b2 = strips[4 * g + j]
                                    nc.tensor.transpose(
                                        tp[:, j * P : (j + 1) * P],
                                        st2[
                                            :, qi * P - b2 : qi * P - b2 + P
                                        ].bitcast(f32),
                                        ident[:],
                                    )
                                dst = row[:, 4 * g * P : 4 * g * P + gw * P]
                                norm_ctr[0] += 1
                                if "norm" in ablate:
                                    pass
                                elif norm_ctr[0] % 10 < ACT_NORM_TENTHS:
                                    nc.scalar.mul(dst, tp[:, 0 : gw * P], rz[:])
                                else:
                                    nc.vector.tensor_scalar_mul(
                                        dst, tp[:, 0 : gw * P], rz[:]
                                    )
                                if g == qi // 4 and "attn_dma" not in ablate:
                                    nc.sync.dma_start(
                                        attn_sh[
                                            h, qi * P : (qi + 1) * P,
                                            0 : (qi + 1) * P,
                                        ],
                                        row[:, 0 : (qi + 1) * P],
                                    )
                            pend.append((h, g, piece))

            # flush remaining deferred attn-row work (tail of the last head)
            for _, _, fn in pend:
                fn()
            pend.clear()
    nc.finalize()
    return nc


@lru_cache(maxsize=1)
def _module():
    return _build()


def _run(in_maps, trace=False):
    nc = _module()
    return run_bass_kernel_spmd(
        nc, in_maps, core_ids=list(range(N_CORES)), trace=trace
    )


def kernel_with_time(x, W_q, W_k, W_v, mask, trace=False):
    x = np.ascontiguousarray(x, dtype=np.float32)
    mask_b = np.ascontiguousarray(mask[0, :P, :P], dtype=np.int32)
    in_maps = []
    for c in range(N_CORES):
        b, h0 = c // 2, (c % 2) * NH
        in_maps.append(
            {
                "x_sh": np.ascontiguousarray(
                    x[b][:, h0 * DH : (h0 + NH) * DH]
                ),
                "mask_blk": mask_b,
            }
        )
    res = _run(in_maps, trace=trace)

    attn = np.empty((B, H, L, L), dtype=np.float32)
    out_blhd = np.empty((B, L, H, DH), dtype=np.float32)
    for c in range(N_CORES):
        b, h0 = c // 2, (c % 2) * NH
        attn[b, h0 : h0 + NH] = res.results[c]["attn_sh"]
        out_blhd[b][:, h0 : h0 + NH] = res.results[c]["out_sh"]
    out = out_blhd.reshape(B, DH, L * H)
    return (out, attn), res.exec_time_ns


def kernel(x, W_q, W_k, W_v, mask):
    (out, attn), _ = kernel_with_time(x, W_q, W_k, W_v, mask)
    return out, attn


if __name__ == "__main__":
    rng = np.random.default_rng(0)
    x = rng.standard_normal((B, L, D)).astype(np.float32)
    w = np.zeros((D, D), np.float32)
    mask = np.broadcast_to(
        np.tril(np.ones((L, L), np.int32)), (B, L, L)
    )
    (out, attn), t = kernel_with_time(x, w, w, w, mask, trace=False)
    print("out", out.shape, "attn", attn.shape, "time", t)



# revision 28
# speedup vs baseline: 1.0055x; 1.0055x over previous
"""Trainium2 Bass kernel for nn_MultiHeadAttention_70497593196785.

Causal MHA with q=k=v=x.view(B,L,H,DH) (the projections are dead code in the
reference). Returns (out, attn) where out is (B, DH, L*H) — a pure reshape of
the (B, L, H, DH) attention output — and attn is the full (B, H, L, L) softmax
matrix.

Sharding: B*H = 64 (batch, head) pairs; core c owns batch c//2 and heads
[(c%2)*8, (c%2)*8+8). Attention is independent per (b, h): no collectives.

Per-core algorithm, per head:
  - xT (64, L) via PE transposes of x blocks; xv = [x | ones] (128-row blocks).
  - For each k-block kb: scoresT[kb] = (xT[:,kb].T @ xT)  (k on partitions,
    q free, lower-triangular region only), exp via ACT (scale=1/8) into an
    SBUF strip, causal masking of the diagonal block via a triu multiply.
  - PV: out.T chunks (65, 512) accumulate xv[kb].T @ strip[kb] over kb; the
    ones column yields the softmax denominator Z per q (flash-style late
    normalization).
  - Per q-block: transpose out.T chunk -> (128, 65), rz = 1/Z, write out rows;
    PE-transpose strip blocks back to (q, k) layout and scale by rz while
    copying PSUM->SBUF (split between DVE and ACT), DMA attn rows.
  - attn's upper triangle is never written: output buffers are zero-initialized
    by the runtime (donated zero buffers), matching exp(-1e9) == 0 exactly.

fp32r (tf32-class PE rounding, ~6e-5 rel err measured, full PE speed) is used
for both matmul stages by default; transposes are plain fp32 (exact).
"""

import math
import os
from functools import lru_cache

import numpy as np

import concourse.bacc as bacc
import concourse.mybir as mybir
import concourse.tile as tile
from concourse.bass_utils import run_bass_kernel_spmd
from concourse.masks import make_identity

f32 = mybir.dt.float32
f32r = mybir.dt.float32r
i32 = mybir.dt.int32
EXP = mybir.ActivationFunctionType.Exp

B, L, D = 4, 2048, 1024
H, DH = 16, 64
P = 128
NH = 8            # heads per core
KB = L // P       # 16 k-blocks
CH = 512          # PV chunk width
NCH = L // CH     # 4 chunks
SCALE = 1.0 / math.sqrt(DH)
N_CORES = 8

# dtype knobs: f32r = fast (tf32-class rounding in PE), f32 = exact, 4x slower
SC_DT = f32r      # QK^T scores matmul
PV_DT = f32r      # attn @ V matmul
# fraction (out of 10) of transposed-attn groups normalized on ScalarE
# instead of VectorE, to balance the two engines
ACT_NORM_TENTHS = 3
SPSUM_COLS = 1024  # exp window width (PSUM banks per S tile = COLS/512)
SPSUM_BUFS = 2
NPOP = 8           # deferred epilogue pieces popped per score slot
XT_NPOP = 0        # pops during the head-start xT phase (hurts: delays xT->scores)
PV_TRIM = True     # skip the zeroed left region of same-chunk PV streams
ROWP_BUFS = 6
ZEROS_ENGINE = "vector"   # or "gpsimd"
TRIU_ENGINE = "vector"    # or "gpsimd"


def _build(ablate=()):
    """ablate: subset of {"attn_dma", "norm", "attn_tr", "exp", "out_dma",
    "scores", "pv"} — timing experiments only (results become wrong)."""
    nc = bacc.Bacc(None, target_bir_lowering=False)
    x_sh = nc.dram_tensor("x_sh", [L, NH * DH], f32, kind="ExternalInput")
    mask_blk = nc.dram_tensor("mask_blk", [P, P], i32, kind="ExternalInput")
    attn_sh = nc.dram_tensor("attn_sh", [NH, L, L], f32, kind="ExternalOutput")
    out_sh = nc.dram_tensor("out_sh", [L, NH, DH], f32, kind="ExternalOutput")

    strip_dt = f32r if (PV_DT == f32r or SC_DT == f32r) else f32

    with tile.TileContext(nc) as tc:
        with (
            tc.tile_pool(name="const", bufs=1) as const,
            tc.tile_pool(name="xvp", bufs=2) as xvp,
            tc.tile_pool(name="xtp", bufs=2) as xtp,
            tc.tile_pool(name="strips", bufs=1) as strips_pool,
            tc.tile_pool(name="spsum", bufs=SPSUM_BUFS, space="PSUM") as spsum,
            tc.tile_pool(name="pvpsum", bufs=1, space="PSUM") as pvpsum,
            tc.tile_pool(name="tpsum", bufs=3, space="PSUM") as tpsum,
            tc.tile_pool(name="rowp", bufs=ROWP_BUFS) as rowp,
            tc.tile_pool(name="otp", bufs=2) as otp,
            tc.tile_pool(name="rzp", bufs=12) as rzp,
            tc.tile_pool(name="outsb", bufs=8) as outsb,
        ):
            ident = const.tile([P, P], f32)
            make_identity(nc, ident)

            # causal masks for the diagonal 128-blocks, from the input mask.
            # tril = mask[0:128, 0:128]; triu (its transpose) masks the
            # diagonal block in (k, q) layout.
            mi = const.tile([P, P], i32, tag="mi")
            nc.sync.dma_start(mi[:], mask_blk[:])
            maskf = const.tile([P, P], f32, tag="maskf")
            nc.vector.tensor_copy(maskf[:], mi[:])
            tmq = tpsum.tile([P, CH], f32, tag="t")
            nc.tensor.transpose(tmq[:, 0:P], maskf[:], ident[:])
            triu = const.tile([P, P], f32, tag="triu")
            nc.vector.tensor_copy(triu[:], tmq[:, 0:P])

            ones_kb = const.tile([P, KB], f32, tag="ones_kb")
            nc.vector.memset(ones_kb[:], 1.0)
            zeros_t = const.tile([P, CH], f32, tag="zeros_t")
            nc.vector.memset(zeros_t[:], 0.0)

            norm_ctr = [0]
            pend = []  # (head, strip_group, closure)
            xv_tiles = {}

            def load_xv(h):
                # Prefetched on the ScalarE HWDGE queue so it is not stuck
                # behind the 1MB attn-row writes on the SP queue.
                xv = xvp.tile([P, KB, DH + 1], PV_DT, tag="xv", name="xv")
                src = x_sh[:].rearrange("(ko p) d -> p ko d", p=P)[
                    :, :, h * DH : (h + 1) * DH
                ]
                if PV_DT != f32:
                    src = src.bitcast(PV_DT)
                nc.scalar.dma_start(xv[:, :, 0:DH], src)
                nc.vector.tensor_copy(xv[:, :, DH], ones_kb[:])
                xv_tiles[h] = xv

            load_xv(0)
            for h in range(NH):
                xv = xv_tiles.pop(h)

                # ---- xT (64, L) via PE transposes, 4 blocks per PSUM tile
                xt = xtp.tile([DH, L], SC_DT, tag="xt")
                for g in range(KB // 4):
                    for _ in range(min(len(pend), XT_NPOP)):
                        pend.pop(0)[2]()
                    tp = tpsum.tile([P, CH], f32, tag="t")
                    for j in range(4):
                        kb = 4 * g + j
                        nc.tensor.transpose(
                            tp[0:DH, j * P : (j + 1) * P],
                            xv[:, kb, 0:DH].bitcast(f32),
                            ident[:],
                        )
                    nc.vector.tensor_copy(
                        xt[:, g * CH : (g + 1) * CH], tp[0:DH, :]
                    )

                # ---- main loop over k-blocks
                strips = []  # (tile, base_col)
                ot = otp.tile([DH + 1, L], f32, tag="ot")
                for kb in range(KB):
                    # Interleave deferred attn-row work with the score
                    # matmuls so PE alternates between PSUM-gated transposes
                    # and deeply-pipelined matmuls. Items from the previous
                    # head MUST pop before the strip tag they read is
                    # rewritten (item g covers strips 4g..4g+3, rewritten by
                    # scores kb>=4g of the next head).
                    nmust = sum(
                        1 for it in pend if it[0] != h and it[1] * 4 <= kb
                    )
                    npop = max(nmust, min(len(pend), NPOP))
                    for _ in range(npop):
                        pend.pop(0)[2]()

                    if kb == 4 and h + 1 < NH:
                        load_xv(h + 1)

                    c0 = kb // 4
                    base = CH * c0
                    slen = L - base
                    st = strips_pool.tile([P, slen], strip_dt, tag=f"st{kb}")
                    lhs = xt[:, kb * P : (kb + 1) * P]
                    off = kb * P - base
                    # scores + exp over [kb*P, L) only (cols left of the
                    # diagonal block are masked anyway), in <=1024 col windows
                    j0 = off
                    while j0 < slen:
                        w = min(SPSUM_COLS, slen - j0)
                        S = spsum.tile([P, SPSUM_COLS], f32, tag="s")
                        for p0 in range(0, w, CH):
                            pw = min(CH, w - p0)
                            nc.tensor.matmul(
                                S[:, p0 : p0 + pw],
                                lhs,
                                xt[:, base + j0 + p0 : base + j0 + p0 + pw],
                                start=True,
                                stop=True,
                            )
                        if "exp" not in ablate:
                            nc.scalar.activation(
                                st[:, j0 : j0 + w], S[:, 0:w], EXP, scale=SCALE
                            )
                        j0 += w
                    # causal: zero columns left of the diagonal block, and
                    # apply the (transposed) triangular mask on the diagonal
                    if off:
                        getattr(nc, ZEROS_ENGINE).tensor_copy(
                            st[:, 0:off], zeros_t[:, 0:off]
                        )
                    getattr(nc, TRIU_ENGINE).tensor_tensor(
                        st[:, off : off + P],
                        st[:, off : off + P].bitcast(f32),
                        triu[:],
                        mybir.AluOpType.mult,
                    )
                    strips.append((st, base))

                    if kb % 4 != 3:
                        continue

                    # ---- PV for chunk c: out.T (65, CH) += xv[k2].T @ strip
                    c = kb // 4
                    pv = pvpsum.tile([P, CH], f32, tag="pv")
                    for k2 in [] if "pv" in ablate else range(kb + 1):
                        st2, b2 = strips[k2]
                        t0 = c * CH - b2
                        d0 = 0
                        if PV_TRIM and k2 // 4 == c and k2 % 4:
                            d0 = (k2 % 4) * P  # cols < k2*P are zero
                        rhs = st2[:, t0 + d0 : t0 + CH]
                        if PV_DT != strip_dt:
                            rhs = rhs.bitcast(PV_DT)
                        nc.tensor.matmul(
                            pv[0 : DH + 1, d0:CH],
                            xv[:, k2, :],
                            rhs,
                            start=(k2 == 0),
                            stop=(k2 == kb),
                        )
                    nc.vector.tensor_copy(
                        ot[:, c * CH : (c + 1) * CH], pv[0 : DH + 1, :]
                    )

                    # ---- per q-block epilogue for this chunk: the out rows
                    # are cheap and emitted inline; the attn-row transposes
                    # and normalizes are queued g-major (g = strip group) and
                    # interleaved into later score slots (see pop loop above).
                    rows = {}
                    for qi in range(4 * c, 4 * c + 4):
                        tq = tpsum.tile([P, CH], f32, tag="t")
                        nc.tensor.transpose(
                            tq[:, 0 : DH + 1],
                            ot[:, qi * P : (qi + 1) * P],
                            ident[0 : DH + 1, 0 : DH + 1],
                        )
                        rz = rzp.tile([P, 1], f32, tag="rz")
                        nc.vector.reciprocal(rz[:], tq[:, DH : DH + 1])
                        ob = outsb.tile([P, DH], f32, tag="ob")
                        nc.vector.tensor_scalar_mul(ob[:], tq[:, 0:DH], rz[:])
                        if "out_dma" not in ablate:
                            nc.sync.dma_start(
                                out_sh[qi * P : (qi + 1) * P, h, :], ob[:]
                            )
                        rows[qi] = (rowp.tile([P, L], f32, tag="row", name="row"), rz)

                    if "attn_tr" in ablate:
                        continue
                    for g in range(0, c + 1):
                        for qi in range(max(4 * c, 4 * g), 4 * c + 4):
                            def piece(g=g, qi=qi, h=h, strips=strips,
                                      row=rows[qi][0], rz=rows[qi][1]):
                                gw = min(4, qi + 1 - 4 * g)
                                tp = tpsum.tile([P, CH], f32, tag="t")
                                for j in range(gw):
                                    st2, b2 = strips[4 * g + j]
                                    nc.tensor.transpose(
                                        tp[:, j * P : (j + 1) * P],
                                        st2[
                                            :, qi * P - b2 : qi * P - b2 + P
                                        ].bitcast(f32),
                                        ident[:],
                                    )
                                dst = row[:, 4 * g * P : 4 * g * P + gw * P]
                                norm_ctr[0] += 1
                                if "norm" in ablate:
                                    pass
                                elif norm_ctr[0] % 10 < ACT_NORM_TENTHS:
                                    nc.scalar.mul(dst, tp[:, 0 : gw * P], rz[:])
                                else:
                                    nc.vector.tensor_scalar_mul(
                                        dst, tp[:, 0 : gw * P], rz[:]
                                    )
                                if g == qi // 4 and "attn_dma" not in ablate:
                                    nc.sync.dma_start(
                                        attn_sh[
                                            h, qi * P : (qi + 1) * P,
                                            0 : (qi + 1) * P,
                                        ],
                                        row[:, 0 : (qi + 1) * P],
                                    )
                            pend.append((h, g, piece))

            # flush remaining deferred attn-row work (tail of the last head)
            for _, _, fn in pend:
                fn()
            pend.clear()
    nc.finalize()
    return nc


@lru_cache(maxsize=1)
def _module():
    return _build()


def _run(in_maps, trace=False):
    nc = _module()
    return run_bass_kernel_spmd(
        nc, in_maps, core_ids=list(range(N_CORES)), trace=trace
    )


def kernel_with_time(x, W_q, W_k, W_v, mask, trace=False):
    x = np.ascontiguousarray(x, dtype=np.float32)
    mask_b = np.ascontiguousarray(mask[0, :P, :P], dtype=np.int32)
    in_maps = []
    for c in range(N_CORES):
        b, h0 = c // 2, (c % 2) * NH
        in_maps.append(
            {
                "x_sh": np.ascontiguousarray(
                    x[b][:, h0 * DH : (h0 + NH) * DH]
                ),
                "mask_blk": mask_b,
            }
        )
    res = _run(in_maps, trace=trace)

    attn = np.empty((B, H, L, L), dtype=np.float32)
    out_blhd = np.empty((B, L, H, DH), dtype=np.float32)
    for c in range(N_CORES):
        b, h0 = c // 2, (c % 2) * NH
        attn[b, h0 : h0 + NH] = res.results[c]["attn_sh"]
        out_blhd[b][:, h0 : h0 + NH] = res.results[c]["out_sh"]
    out = out_blhd.reshape(B, DH, L * H)
    return (out, attn), res.exec_time_ns


def kernel(x, W_q, W_k, W_v, mask):
    (out, attn), _ = kernel_with_time(x, W_q, W_k, W_v, mask)
    return out, attn


if __name__ == "__main__":
    rng = np.random.default_rng(0)
    x = rng.standard_normal((B, L, D)).astype(np.float32)
    w = np.zeros((D, D), np.float32)
    mask = np.broadcast_to(
        np.tril(np.ones((L, L), np.int32)), (B, L, L)
    )
    (out, attn), t = kernel_with_time(x, w, w, w, mask, trace=False)
    print("out", out.shape, "attn", attn.shape, "time", t)

